# revision 14
# baseline (speedup 1.0000x reference)
"""Trainium2 Bass kernel for GQA causal self-attention — fp8 DoubleRow version.

Model (hardcoded): B=2, T=2048, C=2048, n_head=32, n_kv=8, hs=64
Sharding: core g = (batch g//4, head-group g%4), 8 q-heads / 2 kv-heads per core.

Precision plan (error budget 2e-2 max-rel):
 - QKV (token windows 1-3) and c_proj (rows 512+) run in fp8e4m3 with
   DoubleRow perf mode: two 128-deep contraction tiles per instruction at
   0.5 cycles/row (4x bf16).  Weights are host-scaled by 16 to clear the
   e4m3 subnormal floor; inverse scales fold into the psum evacuations.
 - Scores for windows 2-3 run as zero-half DoubleRow on fp8 q/kT twins
   (2x PE rate; the DR pair's second slot points at a zeroed block).
   Window 0-1 scores, softmax and PV stay bf16: raw scores reach ~77
   (structured outliers), so the exp output needs bf16 range.
 - Tier-0 accuracy: fp8 noise is amplified where softmax averages few
   values (sum w^2 ~ 1), i.e. early query tokens.  Tokens 0:512 (= window
   0) get a fully bf16 path: window-0 projections from bf16 inputs, bf16
   attention (the normal code path), and bf16 c_proj for output rows
   0:512.  Later windows ride the fp8 projections; their y is quantized to
   fp8 (x8 scale) for the DR c_proj.

Layout/schedule bones are inherited from the bf16 kernel: K-stationary
scores (S^T tiles), pair-of-heads per matmul on PE rows 0:63/64:127, PV
with a ones-column for the softmax denominator, block-causal skipping, and
a micro-task queue that feeds projection/c_proj fillers to the PE between
attention steps.
"""

import sys
from collections import deque

import numpy as np
import ml_dtypes
from contextlib import ExitStack

for _p in ("/opt/trn_rl_repo", "/root/.axon_site/_ro/trn_rl_repo"):
    if _p not in sys.path:
        sys.path.append(_p)

import concourse.mybir as mybir
import concourse.tile as tile
from concourse import bacc
from concourse.bass_utils import run_bass_kernel_spmd

BF16 = mybir.dt.bfloat16
F32 = mybir.dt.float32
E4 = mybir.dt.float8e4
I16 = mybir.dt.int16
NPBF16 = ml_dtypes.bfloat16
NPE4 = ml_dtypes.float8_e4m3
DR = mybir.MatmulPerfMode.DoubleRow
LOG2E = 1.4426950408889634
# one-op Schraudolph exp on DVE: bf16 bits = s*128*log2e + (16256 - corr + .5)
SCH_A = 128.0 * LOG2E
SCH_B = 16256.0 - 0.458 * 128.0 + 0.5

B, T, C = 2, 2048, 2048
N_HEAD, N_KV, HS = 32, 8, 64
NE = 2048
N_CORES = 8
HL = 8          # q heads per core
KVL = 2         # kv heads per core
P = 128
TQ = 512        # tq tile (matmul moving width)
NJ = T // TQ    # 4 tq tiles
NT = T // P     # 16 token tiles
KC = C // P     # 16 contraction tiles over channels
QROWS = HL * HS          # 512 local q rows
KROWS = KVL * HS         # 128 local k rows
WCOLS = QROWS + 2 * KROWS  # 768 local w_attn rows
SW = 16.0       # host scale on w_attn / w_proj (fp8 subnormal avoidance)
SY = 8.0        # on-device scale on y before fp8

# position-block -> local head: q_sb m-tile mt rows [0:64]=head mt, [64:128]=head mt+4
Q_ORDER = [0, 4, 1, 5, 2, 6, 3, 7]

_CACHE = {}


def _build_program():
    nc = bacc.Bacc("TRN2", target_bir_lowering=False, debug=False)

    xT8_d = nc.dram_tensor("xT8", [C, T - TQ], E4, kind="ExternalInput")
    x0bf_d = nc.dram_tensor("x0bf", [C, TQ], BF16, kind="ExternalInput")
    w8_d = nc.dram_tensor("w8", [C, WCOLS], E4, kind="ExternalInput")
    wbf_d = nc.dram_tensor("wbf", [C, WCOLS], BF16, kind="ExternalInput")
    wp8_d = nc.dram_tensor("wp8", [QROWS, C], E4, kind="ExternalInput")
    wpbf_d = nc.dram_tensor("wpbf", [QROWS, C], BF16, kind="ExternalInput")
    bqk_d = nc.dram_tensor("bqk", [6, P], F32, kind="ExternalInput")
    out_d = nc.dram_tensor("out", [T, C], BF16, kind="ExternalOutput")
    # window-3 c_proj partial over k-tile pair (2,3) (head-pairs 2,3): summed
    # into out on the host.  Splitting it out lets the (0,1) pair of window
    # 3's c_proj overlap the last attention pairs instead of serializing.
    out2_d = nc.dram_tensor("out2", [TQ, C], BF16, kind="ExternalOutput")

    with tile.TileContext(nc) as tc:
        with ExitStack() as ctx:
            _emit(ctx, tc, nc, xT8_d, x0bf_d, w8_d, wbf_d, wp8_d, wpbf_d,
                  bqk_d, out_d, out2_d)
    nc.compile()
    return nc


def _emit(ctx, tc, nc, xT8_d, x0bf_d, w8_d, wbf_d, wp8_d, wpbf_d,
          bqk_d, out_d, out2_d):
    ExpF = mybir.ActivationFunctionType.Exp
    CopyF = mybir.ActivationFunctionType.Copy
    add = mybir.AluOpType.add
    mult = mybir.AluOpType.mult

    persist = ctx.enter_context(tc.tile_pool(name="persist", bufs=1))
    # PSUM budget (8 banks):
    #  pps: scores [128,1024] f32 (2 banks) x2 bufs          = 4 banks
    #  ppa: proj/cproj [128,512] f32 (1 bank) x2 bufs        = 2 banks
    #  ppo: PV accum [128,260] f32 / transpose [128,128]     = 2 banks
    pps = ctx.enter_context(tc.tile_pool(name="pps", bufs=2, space="PSUM"))
    ppa = ctx.enter_context(tc.tile_pool(name="ppa", bufs=2, space="PSUM"))
    ppo = ctx.enter_context(tc.tile_pool(name="ppo", bufs=2, space="PSUM"))
    ptpool = ctx.enter_context(tc.tile_pool(name="pt", bufs=9))
    rcpool = ctx.enter_context(tc.tile_pool(name="rc", bufs=12))
    ytpool = ctx.enter_context(tc.tile_pool(name="yt", bufs=6))
    mkpool = ctx.enter_context(tc.tile_pool(name="mk", bufs=2))
    outpool = ctx.enter_context(tc.tile_pool(name="os", bufs=8))

    # ---- persistent SBUF tensors ----
    xT8_sb = persist.tile([P, KC * (T - TQ)], E4, tag="xT8")
    x0_sb = persist.tile([P, KC * TQ], BF16, tag="x0")
    w8_sb = persist.tile([P, KC * WCOLS], E4, tag="w8")
    wbf_sb = persist.tile([P, KC * WCOLS], BF16, tag="wbf")
    wp8_sb = persist.tile([P, 4 * C], E4, tag="wp8")
    wpbf_sb = persist.tile([P, 4 * C], BF16, tag="wpbf")
    q_sb = persist.tile([P, 4 * T], BF16, tag="q")
    kT_sb = persist.tile([P, T], BF16, tag="k")
    # fp8 twins for the windows-2/3 zero-half DoubleRow scores (2x PE rate):
    # kT8 interleaves each [128,128] k-tile with a zero block (the DR
    # stationary's second slot); q8 holds windows 2-3 only, one real TQ
    # block + one zero TQ block per (mt, window) for the moving operand.
    kT8_sb = persist.tile([P, 2 * T], E4, tag="k8")
    q8_sb = persist.tile([P, 8 * 2 * TQ], E4, tag="q8")
    sc125 = persist.tile([P, 1], F32, tag="sc125")
    v_sb = persist.tile([P, NT * 130], BF16, tag="v")
    y8_sb = persist.tile([P, 4 * T], E4, tag="y8")
    y0_sb = persist.tile([P, 4 * TQ], BF16, tag="y0")
    bqk_sb = persist.tile([P, 6], F32, tag="bqk")
    ident = persist.tile([P, P], BF16, tag="ident")
    # single triangular mask: maskt[x, y] = 1 if y >= x else 0 (bf16)
    maskt = persist.tile([P, P], BF16, tag="maskt")

    # ---- input DMAs ----
    # dma_start costs ~650ns on the issuing sequencer and ~630ns on HWDGE,
    # so transfers are merged into few issues, emitted in consumption order.
    # The stream is sliced by priority: window-0's kT/v/q0 inputs first (ACT
    # exp work — the bottleneck — starts with the first attention pair),
    # then the q-mt1-3 weight slices, then window-1's fp8 inputs, etc.
    x0v = x0bf_d.ap().rearrange("(k p) t -> p k t", p=P)
    x0s = x0_sb.rearrange("p (k t) -> p k t", t=TQ)
    wbv = wbf_d.ap().rearrange("(k p) c -> p k c", p=P)
    wbs = wbf_sb.rearrange("p (k c) -> p k c", c=WCOLS)
    nc.sync.dma_start(x0s[:, 0:2, :], x0v[:, 0:2, :])
    nc.sync.dma_start(wbs[:, 0:2, QROWS:], wbv[:, 0:2, QROWS:])
    nc.sync.dma_start(wbs[:, 0:2, 0:P], wbv[:, 0:2, 0:P])
    for k0, k1 in ((2, 6), (6, 11), (11, 16)):
        nc.sync.dma_start(x0s[:, k0:k1, :], x0v[:, k0:k1, :])
        nc.sync.dma_start(wbs[:, k0:k1, QROWS:], wbv[:, k0:k1, QROWS:])
        nc.sync.dma_start(wbs[:, k0:k1, 0:P], wbv[:, k0:k1, 0:P])
    # biases (needed only at first evac, ~13us in)
    nc.sync.dma_start(bqk_sb[:], bqk_d.ap().rearrange("t p -> p t"))
    # q-mt1..3 weight slices (window-0 pairs 1-3)
    for k0, k1 in ((0, 8), (8, 16)):
        nc.sync.dma_start(wbs[:, k0:k1, P:QROWS], wbv[:, k0:k1, P:QROWS])
    # fp8 inputs for windows 1-3 (xT8 column n is token TQ+n), again sliced
    # so window-1's first pair (k/q0/v units) unblocks earliest
    xv8 = xT8_d.ap().rearrange("(k p) t -> p k t", p=P)
    wv8 = w8_d.ap().rearrange("(k p) c -> p k c", p=P)
    xs8 = xT8_sb.rearrange("p (k t) -> p k t", t=T - TQ)
    ws8 = w8_sb.rearrange("p (k c) -> p k c", c=WCOLS)
    nc.sync.dma_start(ws8[:, :, QROWS:], wv8[:, :, QROWS:])
    for k0, k1 in ((0, 8), (8, 16)):
        nc.sync.dma_start(xs8[:, k0:k1, 0:TQ], xv8[:, k0:k1, 0:TQ])
    nc.sync.dma_start(ws8[:, :, 0:P], wv8[:, :, 0:P])
    nc.sync.dma_start(ws8[:, :, P:QROWS], wv8[:, :, P:QROWS])
    for n in range(2, NJ):
        t0, t1 = (n - 1) * TQ, n * TQ
        for k0, k1 in ((0, 8), (8, 16)):
            nc.sync.dma_start(xs8[:, k0:k1, t0:t1], xv8[:, k0:k1, t0:t1])
    wpv8 = wp8_d.ap().rearrange("(k p) c -> p k c", p=P)
    wps8 = wp8_sb.rearrange("p (k c) -> p k c", c=C)
    for k0, k1 in ((0, 2), (2, 4)):
        nc.sync.dma_start(wps8[:, k0:k1, :], wpv8[:, k0:k1, :])
    wpvb = wpbf_d.ap().rearrange("(k p) c -> p k c", p=P)
    wpsb = wpbf_sb.rearrange("p (k c) -> p k c", c=C)
    for k0, k1 in ((0, 2), (2, 4)):
        nc.sync.dma_start(wpsb[:, k0:k1, :], wpvb[:, k0:k1, :])

    # ---- constants ----
    # warm the PE p-state ramp during the initial DMA wait (~3us of dummy
    # matmuls so the first real matmul runs at 2.4GHz).
    dmy = persist.tile([P, P], BF16, tag="dmy")
    nc.gpsimd.memset(dmy[:], 0.0)
    for w in range(26):
        pdmy = ppa.tile([P, P], F32, tag="pa", name="pdmy")
        nc.tensor.matmul(pdmy[:], dmy[:, 0:P], dmy[:, 0:P],
                         start=True, stop=True)
    mf = mkpool.tile([P, P], F32, tag="mf")
    nc.gpsimd.memset(mf[:], 1.0)
    nc.gpsimd.affine_select(
        out=mf[:], in_=mf[:], compare_op=mybir.AluOpType.is_ge,
        fill=0.0, base=0, pattern=[[1, P]], channel_multiplier=-1)
    nc.scalar.copy(maskt[:], mf[:])
    # identity for PE transposes
    mi = mkpool.tile([P, P], F32, tag="mi")
    nc.gpsimd.memset(mi[:], 1.0)
    nc.gpsimd.affine_select(
        out=mi[:], in_=mi[:], compare_op=mybir.AluOpType.is_equal,
        fill=0.0, base=0, pattern=[[1, P]], channel_multiplier=-1)
    nc.scalar.copy(ident[:], mi[:])
    nc.vector.memset(v_sb[:], 1.0)   # ones columns; data cols overwritten
    nc.vector.memset(sc125[:], 0.125)
    # zero the fp8 score-twin tensors (zero-half DR slots) on idle Pool
    nc.gpsimd.memset(kT8_sb[:], 0.0)
    nc.gpsimd.memset(q8_sb[:], 0.0)

    def xt8(kp, c0, n):    # xT8 k-pair AP [128, 2, n]; c0 in 0..T-TQ-1
        return xs8[:, kp:kp + 2, c0:c0 + n]

    def wq8(kp, mt):       # [128, 2, 128] q-weight pair (fp8)
        return ws8[:, kp:kp + 2, mt * P:(mt + 1) * P]

    def wk8(kp):
        return ws8[:, kp:kp + 2, QROWS:QROWS + P]

    def wv8t(kp):
        return ws8[:, kp:kp + 2, QROWS + P:QROWS + 2 * P]

    kT8v = kT8_sb.rearrange("p (i two c) -> p i two c", two=2, c=P)
    q8v = q8_sb.rearrange("p (g two t) -> p g two t", two=2, t=TQ)

    def x0t(k, c0, n):     # window-0 bf16 x tile
        return x0_sb[:, k * TQ + c0:k * TQ + c0 + n]

    def wbf_t(k, m0, n):   # bf16 w slice
        return wbf_sb[:, k * WCOLS + m0:k * WCOLS + m0 + n]

    # ---- micro-task queue ------------------------------------------------
    queue = deque()
    drain_mode = [False]
    drain_flip = [0]
    drain_pool = [0]
    vc = {"pe": 0.0, "act": 0.0}

    def pe_adv(ns):
        vc["pe"] += ns

    def pull(ns):
        while ns > 0 and queue:
            pe_ns, fn = queue.popleft()
            if fn is None:
                continue
            fn()
            vc["pe"] += pe_ns
            ns -= pe_ns

    def pull_to_act(margin=0.0):
        while vc["pe"] < vc["act"] + margin and queue:
            pe_ns, fn = queue.popleft()
            if fn is None:
                continue
            fn()
            vc["pe"] += pe_ns

    def drain():
        pull(float("inf"))

    def drain_to_marker():
        # pull until the next marker (end of a proj group) is consumed;
        # later fillers stay queued for in-window pulls
        while queue:
            pe_ns, fn = queue.popleft()
            if fn is None:
                return
            fn()
            vc["pe"] += pe_ns


    # ---- work units ------------------------------------------------------
    def unit_q(n, mt, pool=None, run=False):
        # fp8 DR q projection for window n>=1
        pool = pool or ppa
        st = {}
        micros = []
        c0 = (n - 1) * TQ

        def m_first():
            st["ps"] = pool.tile([P, TQ], F32, tag=_ptag(pool), name="psq")
            for kp in (0, 2, 4, 6):
                nc.tensor.matmul(st["ps"][:], wq8(kp, mt), xt8(kp, c0, TQ),
                                 start=(kp == 0), stop=False, perf_mode=DR)
        micros.append((430, m_first))

        def m_second():
            for kp in (8, 10, 12, 14):
                nc.tensor.matmul(st["ps"][:], wq8(kp, mt), xt8(kp, c0, TQ),
                                 start=False, stop=(kp == 14), perf_mode=DR)
            if n >= 2:
                nc.vector.tensor_scalar(
                    out=q8v[:, mt * 2 + (n - 2), 0, :],
                    in0=st["ps"][:], scalar1=1.0 / SW,
                    scalar2=bqk_sb[:, mt:mt + 1], op0=mult, op1=add)
            else:
                nc.vector.tensor_scalar(
                    out=q_sb[:, mt * T + n * TQ: mt * T + (n + 1) * TQ],
                    in0=st["ps"][:], scalar1=1.0 / SW,
                    scalar2=bqk_sb[:, mt:mt + 1], op0=mult, op1=add)
        micros.append((430, m_second))
        return _unit_done(micros, run)

    def unit_k(n, pool=None, run=False):
        pool = pool or ppa
        st = {}
        micros = []
        c0 = (n - 1) * TQ

        def m_first():
            st["ps"] = pool.tile([P, TQ], F32, tag=_ptag(pool), name="psk")
            for kp in (0, 2, 4, 6):
                nc.tensor.matmul(st["ps"][:], wk8(kp), xt8(kp, c0, TQ),
                                 start=(kp == 0), stop=False, perf_mode=DR)
        micros.append((430, m_first))

        def m_second():
            for kp in (8, 10, 12, 14):
                nc.tensor.matmul(st["ps"][:], wk8(kp), xt8(kp, c0, TQ),
                                 start=False, stop=(kp == 14), perf_mode=DR)
            nc.vector.tensor_scalar(
                out=kT_sb[:, n * TQ:(n + 1) * TQ],
                in0=st["ps"][:], scalar1=0.125 / SW, scalar2=bqk_sb[:, 4:5],
                op0=mult, op1=add)
            nc.vector.tensor_scalar(
                out=kT8v[:, 4 * n:4 * n + 4, 0, :],
                in0=st["ps"].rearrange("p (f c) -> p f c", c=P),
                scalar1=1.0 / SW, scalar2=bqk_sb[:, 5:6],
                op0=mult, op1=add)
        micros.append((430, m_second))
        return _unit_done(micros, run)

    def unit_v(i, pool=None, run=False):
        # v_sb tile i: [0:64]=kv0, 64=ones, [65:129]=kv1, 129=ones
        pool = pool or ppa
        micros = []
        c0 = (i - 4) * P

        def m_all():
            ps = pool.tile([P, P], F32, tag=_ptag(pool), name="psv")
            for kp in (0, 2, 4, 6, 8, 10, 12, 14):
                nc.tensor.matmul(ps[:, 0:P], xt8(kp, c0, P), wv8t(kp),
                                 start=(kp == 0), stop=(kp == 14),
                                 perf_mode=DR)
            nc.vector.tensor_scalar(out=v_sb[:, i * 130: i * 130 + 64],
                                    in0=ps[:, 0:64], scalar1=1.0 / SW,
                                    scalar2=None, op0=mult)
            nc.vector.tensor_scalar(out=v_sb[:, i * 130 + 65: i * 130 + 129],
                                    in0=ps[:, 64:128], scalar1=1.0 / SW,
                                    scalar2=None, op0=mult)
        micros.append((215, m_all))
        return _unit_done(micros, run)

    def unit_cproj(j, ms, run=False, ks=(0, 1, 2, 3), dst=None):
        # fp8 DR c_proj (j>=1): k-tile pairs (0,1)/(2,3); ks is (0,1,2,3),
        # (0,1) or (2,3)
        dst = dst if dst is not None else out_d
        row0 = (j * TQ if dst is out_d else 0) + ms * P
        y8v = y8_sb.rearrange("p (k t) -> p k t", t=T)
        wv = wp8_sb.rearrange("p (k c) -> p k c", c=C)
        pairs = [kp for kp in (0, 2) if kp in ks]
        micros = []
        ust = {}
        for n in range(NJ):
            st = {}

            def m_a(n=n, st=st):
                if drain_mode[0]:
                    drain_pool[0] ^= 1
                pool, tg = (pps, "ps") if drain_pool[0] else (ppa, "pa")
                st["pc"] = pool.tile([P, TQ], F32, tag=tg, name="pc")
                for pi, kp in enumerate(pairs):
                    nc.tensor.matmul(
                        st["pc"][:],
                        y8v[:, kp:kp + 2, j * TQ + ms * P: j * TQ + (ms + 1) * P],
                        wv[:, kp:kp + 2, n * TQ:(n + 1) * TQ],
                        start=(pi == 0), stop=(pi == len(pairs) - 1),
                        perf_mode=DR)
                _cproj_out(ust, st, n, dst, row0, 1.0 / (SY * SW))
            micros.append((108 * len(pairs), m_a))
        return _unit_done(micros, run)

    def unit_cproj0(j, ms, run=False):
        # bf16 c_proj for window-0 output rows (from y0_sb / wpbf)
        row0 = j * TQ + ms * P
        micros = []
        ust = {}
        for n in range(NJ):
            st = {}

            def m_a(n=n, st=st):
                if drain_mode[0]:
                    drain_pool[0] ^= 1
                pool, tg = (pps, "ps") if drain_pool[0] else (ppa, "pa")
                st["pc"] = pool.tile([P, TQ], F32, tag=tg, name="pc0")
                for k in (0, 1):
                    nc.tensor.matmul(
                        st["pc"][:], y0_sb[:, k * TQ + ms * P:k * TQ + (ms + 1) * P],
                        wpbf_sb[:, k * C + n * TQ:k * C + (n + 1) * TQ],
                        start=(k == 0), stop=False)

            def m_b(n=n, st=st):
                for k in (2, 3):
                    nc.tensor.matmul(
                        st["pc"][:], y0_sb[:, k * TQ + ms * P:k * TQ + (ms + 1) * P],
                        wpbf_sb[:, k * C + n * TQ:k * C + (n + 1) * TQ],
                        start=False, stop=(k == 3))
                _cproj_out(ust, st, n, out_d, row0, 1.0 / SY)
            micros.append((215, m_a))
            micros.append((215, m_b))
        return _unit_done(micros, run)

    def _cproj_out(ust, st, n, dst, row0, scl):
        # evacuate into a unit-wide staging tile; one bundled DMA per unit
        if n == 0:
            ust["os"] = outpool.tile([P, NJ * TQ], BF16, tag="os", name="os")
        drain_flip[0] ^= 1
        if drain_mode[0] and drain_flip[0]:
            # post-attention drain: ACT is idle; alternate psum evacuation
            # between DVE/ACT so the 2-slot psum ring doesn't serialize
            nc.scalar.activation(ust["os"][:, n * TQ:(n + 1) * TQ],
                                 st["pc"][:], CopyF, scale=scl)
        else:
            nc.vector.tensor_scalar(
                out=ust["os"][:, n * TQ:(n + 1) * TQ], in0=st["pc"][:],
                scalar1=scl, scalar2=None, op0=mult)
        if drain_mode[0] and n == 1:
            nc.scalar.dma_start(dst.ap()[row0: row0 + P, 0:2 * TQ],
                                ust["os"][:, 0:2 * TQ])
        elif drain_mode[0] and n == NJ - 1:
            nc.sync.dma_start(dst.ap()[row0: row0 + P, 2 * TQ:],
                              ust["os"][:, 2 * TQ:])
        elif n == NJ - 1:
            nc.sync.dma_start(dst.ap()[row0: row0 + P, :], ust["os"][:])

    def _ptag(pool):
        return {id(pps): "ps", id(ppa): "pa", id(ppo): "po"}[id(pool)]

    def _unit_done(micros, run):
        if run:
            for _, fn in micros:
                fn()
        else:
            queue.extend(micros)

    def _offload_exp(j, hp, i):
        # DVE-Schraudolph exp offload: disabled — measured error cost
        # (~1e-2, systematic linear-mantissa distortion) eats the budget
        return False

    # ---- attention -------------------------------------------------------
    def emit_score_mm(j, hp, i):
        # S^T tile [tk, tq] for tk-tile i of head pair hp, window j.
        # Head A occupies psum cols [c0:TQ], head B [TQ:2TQ-c0] (shifted
        # left by c0 so the pair-wide exp covers no dead columns).
        r = i - 4 * j
        c0 = max(0, r) * P
        qcol = hp * T + j * TQ
        ps = pps.tile([P, 2 * TQ], F32, tag="ps", name="pss")
        if j >= 2:
            # zero-half DoubleRow on the fp8 twins: 0.5 cycles/row
            g = hp * 2 + (j - 2)
            for h in (0, 1):
                rb = 64 * h
                nc.tensor.matmul(
                    ps[:, (c0 if h == 0 else TQ):
                       (TQ if h == 0 else 2 * TQ - c0)],
                    kT8v[rb:rb + 64, i, :, :],
                    q8v[rb:rb + 64, g, :, c0:TQ],
                    start=True, stop=True, perf_mode=DR)
            pe_adv(2 * (TQ - c0) * 0.2083)
        else:
            for h in (0, 1):
                rb = 64 * h
                nc.tensor.matmul(
                    ps[:, (c0 if h == 0 else TQ):
                       (TQ if h == 0 else 2 * TQ - c0)],
                    kT_sb[rb:rb + 64, i * P:(i + 1) * P],
                    q_sb[rb:rb + 64, qcol + c0: qcol + TQ],
                    start=True, stop=True)
            pe_adv(2 * (TQ - c0) * 0.417)
        return ps

    def attention(j, hp, pre_ps=None):
        # head pair (hp, hp+4): same q/y column tile `hp`, head A on
        # partitions 0:64 (kv0), head B on 64:128 (kv1).  Score matmuls are
        # software-pipelined one tile ahead of the exps so ACT (the
        # bottleneck engine) never waits on the PE; `pre_ps` carries the
        # pipelining across head-pair boundaries.
        nb = 4 * (j + 1)   # tk tiles in play (block-causal)
        mt = hp
        qcol = mt * T + j * TQ
        po = [ppo.tile([P, 260], F32, tag="po", name=f"po{g}")
              for g in range(2)]
        yts = []
        pres = list(pre_ps) if pre_ps else []
        ps_cur = pres.pop(0) if pres else emit_score_mm(j, hp, 0)
        ps_next_pair = []
        for i in range(nb):
            r = i - 4 * j
            roff = max(0, r)
            c0 = roff * P

            def bcol(h, c):
                # pt/psum column of chunk c for head h (B-shifted layout)
                return c * P if h == 0 else TQ + (c - roff) * P

            ps = ps_cur
            pt = ptpool.tile([P, 2 * TQ], BF16, tag="pt", name="pt")
            off = r < 0 and _offload_exp(j, hp, i)
            if off:
                # exp offloaded to DVE as a one-op Schraudolph to bf16 bits
                # (~3% weight error on this tile; ACT is the bottleneck)
                nc.vector.tensor_scalar(
                    out=pt[:, 0:2 * TQ].bitcast(I16), in0=ps[:, 0:2 * TQ],
                    scalar1=SCH_A, scalar2=SCH_B, op0=mult, op1=add)
            elif j >= 2:
                nc.scalar.activation(pt[:, c0:2 * TQ - c0],
                                     ps[:, c0:2 * TQ - c0], ExpF,
                                     scale=sc125[:, 0:1])
            else:
                nc.scalar.activation(pt[:, c0:2 * TQ - c0],
                                     ps[:, c0:2 * TQ - c0], ExpF)
            if i + 1 < nb:
                ps_cur = pres.pop(0) if pres else emit_score_mm(j, hp, i + 1)
            elif hp < 3:
                ps_next_pair.append(emit_score_mm(j, hp + 1, 0))
                ps_next_pair.append(emit_score_mm(j, hp + 1, 1))
            if r >= 0:
                # diagonal boundary chunk: mask the [128,128] block per head
                # (on Pool: it is otherwise idle, and the masked chunk is
                # scheduled last in the PV order so the latency hides)
                for h in (0, 1):
                    nc.vector.tensor_tensor(
                        out=pt[:, bcol(h, r): bcol(h, r) + P],
                        in0=pt[:, bcol(h, r): bcol(h, r) + P],
                        in1=maskt[:], op=mult)
            if i == 0:
                cs = list(range(4))
            elif r >= 0:
                cs = list(range(r + 1, 4)) + [r]
            else:
                cs = list(range(4))
            for c in cs:
                cb = 130 * (c % 2)
                for h in (0, 1):
                    nc.tensor.matmul(
                        po[c // 2][:, cb + 65 * h: cb + 65 * h + 65],
                        pt[:, bcol(h, c): bcol(h, c) + P],
                        v_sb[:, i * 130 + 65 * h: i * 130 + 65 * h + 65],
                        start=(i == 0 and h == 0 and c in (0, 2)),
                        stop=(i == 4 * j + c), skip_group_check=True)
            if r >= 0:
                # chunk r's accumulation just stopped: normalize now so the
                # pair-end transposes only wait on the last chunk.
                yt = ytpool.tile([P, P], BF16, tag="yt", name="yt")
                cb = 130 * (r % 2)
                for h in (0, 1):
                    r_t = rcpool.tile([P, 1], F32, tag="rc", name="rc")
                    nc.vector.reciprocal(
                        r_t[:],
                        po[r // 2][:, cb + 65 * h + 64: cb + 65 * h + 65])
                    nc.vector.tensor_scalar(
                        out=yt[:, 64 * h:64 * h + 64],
                        in0=po[r // 2][:, cb + 65 * h: cb + 65 * h + 64],
                        scalar1=r_t[:, 0:1], scalar2=SY, op0=mult, op1=mult)
                yts.append(yt)
            if not off:
                vc["act"] = max(vc["act"], vc["pe"]) \
                    + (2 * TQ - 2 * c0) * 0.834 + 190
                vc["pe"] = max(vc["pe"], vc["act"])
            pe_adv((4 - max(0, r)) * 2 * 65 * 0.417)
            pull_to_act(500)
        pull(1400)
        # transpose [q, dA|dB] -> [dA|dB, q] into the c_proj layout; the
        # psum->sbuf copy converts bf16 -> fp8 for windows 1-3
        for c in range(4):
            tr = ppo.tile([P, P], BF16, tag="po", name="tr")
            nc.tensor.transpose(tr[:], yts[c][:], ident[:])
            if j == 0:
                nc.vector.tensor_copy(
                    y0_sb[:, mt * TQ + c * P: mt * TQ + (c + 1) * P], tr[:])
            else:
                nc.vector.tensor_copy(
                    y8_sb[:, qcol + c * P: qcol + (c + 1) * P], tr[:])
            if c < 3:
                pull(500)
        return ps_next_pair or None

    def unit_kbf(run=False):
        # accurate bf16 kT for tiles 0-3 (overwrites the fp8-derived boot
        # values once window 1 is done with them)
        st = {}
        micros = []

        def m_first():
            st["ps"] = ppa.tile([P, TQ], F32, tag="pa", name="pskb")
            for k in range(8):
                nc.tensor.matmul(st["ps"][:], wbf_t(k, QROWS, P),
                                 x0t(k, 0, TQ), start=(k == 0), stop=False)
        micros.append((430, m_first))

        def m_second():
            for k in range(8, KC):
                nc.tensor.matmul(st["ps"][:], wbf_t(k, QROWS, P),
                                 x0t(k, 0, TQ), start=False,
                                 stop=(k == KC - 1))
            nc.vector.tensor_scalar(
                out=kT_sb[:, 0:TQ], in0=st["ps"][:], scalar1=0.125,
                scalar2=bqk_sb[:, 4:5], op0=mult, op1=add)
            nc.vector.tensor_scalar(
                out=kT8v[:, 0:4, 0, :],
                in0=st["ps"].rearrange("p (f c) -> p f c", c=P),
                scalar1=bqk_sb[:, 5:6], scalar2=None, op0=add)
        micros.append((430, m_second))
        return _unit_done(micros, run)

    def unit_vbf(i, run=False):
        # accurate bf16 v tile i (0..3), overwriting the boot values
        micros = []

        def m_all():
            ps = ppa.tile([P, TQ], F32, tag="pa", name="psvb")
            for k in range(KC):
                nc.tensor.matmul(ps[:, 0:P], x0t(k, i * P, P),
                                 wbf_t(k, QROWS + P, P),
                                 start=(k == 0), stop=(k == KC - 1))
            nc.vector.tensor_copy(v_sb[:, i * 130: i * 130 + 64], ps[:, 0:64])
            nc.vector.tensor_copy(v_sb[:, i * 130 + 65: i * 130 + 129],
                                  ps[:, 64:128])
        micros.append((860, m_all))
        return _unit_done(micros, run)

    def unit_q0bf(mt, run=False):
        # window-0 bf16 q projection for m-tile mt (deferred from p0 so the
        # first attention pair starts as soon as q-mt0/kT/v are ready)
        st = {}
        micros = []

        def m_first():
            st["ps"] = ppa.tile([P, TQ], F32, tag="pa", name="psq0")
            for k in range(8):
                nc.tensor.matmul(st["ps"][:], wbf_t(k, mt * P, P),
                                 x0t(k, 0, TQ), start=(k == 0), stop=False)
        micros.append((430, m_first))

        def m_second():
            for k in range(8, KC):
                nc.tensor.matmul(st["ps"][:], wbf_t(k, mt * P, P),
                                 x0t(k, 0, TQ), start=False,
                                 stop=(k == KC - 1))
            nc.vector.tensor_scalar(
                out=q_sb[:, mt * T: mt * T + TQ], in0=st["ps"][:],
                scalar1=bqk_sb[:, mt:mt + 1], scalar2=None, op0=add)
        micros.append((430, m_second))
        return _unit_done(micros, run)

    # ---- schedule --------------------------------------------------------
    # p0: window-0 bf16 kT/q-mt0/v projections run directly, k-MAJOR so the
    # PE consumes input chunks in DMA-arrival order; q m-tiles 1-3 are
    # deferred into the filler queue so attention (and with it the ACT exp
    # stream, the bottleneck) starts as early as possible.
    p0q0 = pps.tile([P, TQ], F32, tag="ps", name="p0q0")
    p0k = ppo.tile([P, TQ], F32, tag="po", name="p0k")
    for k0 in range(0, KC, 2):
        for k in (k0, k0 + 1):
            nc.tensor.matmul(p0q0[:], wbf_t(k, 0, P), x0t(k, 0, TQ),
                             start=(k == 0), stop=(k == KC - 1))
        for k in (k0, k0 + 1):
            nc.tensor.matmul(p0k[:], wbf_t(k, QROWS, P), x0t(k, 0, TQ),
                             start=(k == 0), stop=(k == KC - 1))
    nc.vector.tensor_scalar(
        out=q_sb[:, 0:TQ], in0=p0q0[:],
        scalar1=bqk_sb[:, 0:1], scalar2=None, op0=add)
    nc.vector.tensor_scalar(
        out=kT_sb[:, 0:TQ], in0=p0k[:], scalar1=0.125,
        scalar2=bqk_sb[:, 4:5], op0=mult, op1=add)
    nc.vector.tensor_scalar(
        out=kT8v[:, 0:4, 0, :],
        in0=p0k.rearrange("p (f c) -> p f c", c=P),
        scalar1=bqk_sb[:, 5:6], scalar2=None, op0=add)
    # window-0 v tiles (bf16)
    for i, pool in zip(range(4), (ppo, pps, ppa, ppo)):
        ps = pool.tile([P, TQ], F32, tag=_ptag(pool), name="psv0")
        for k in range(KC):
            nc.tensor.matmul(ps[:, 0:P], x0t(k, i * P, P),
                             wbf_t(k, QROWS + P, P),
                             start=(k == 0), stop=(k == KC - 1))
        nc.vector.tensor_copy(v_sb[:, i * 130: i * 130 + 64], ps[:, 0:64])
        nc.vector.tensor_copy(v_sb[:, i * 130 + 65: i * 130 + 129],
                              ps[:, 64:128])

    def queue_proj(n):
        # kT / q-mt0 / v first: the next window's first attention pair only
        # needs those
        unit_k(n)
        unit_q(n, 0)
        for i in range(4 * n, 4 * n + 4):
            unit_v(i)
        queue.append((0, None))     # marker: first-pair prerequisites done
        for mt in (1, 2, 3):
            unit_q(n, mt)

    for mt in (1, 2, 3):
        unit_q0bf(mt)
    for j in range(NJ):
        if j > 0:
            # pull until this window's first-pair prerequisites (kT/q-mt0/v
            # of proj(j)) are emitted; q-mt1..3 stay queued for in-window
            # pulls
            drain_to_marker()
        if j == 3:
            for ms in range(4):
                unit_cproj0(0, ms)
            for ms in range(4):
                unit_cproj(2, ms)
        pre = None
        for hp in range(4):
            pre = attention(j, hp, pre_ps=pre)
            # fillers are queued only once their DMAs are in flight, so a
            # pulled micro never stalls the in-order PE stream (which would
            # stall the next score matmul and starve ACT)
            if j == 0 and hp == 2:
                queue_proj(1)
            elif j == 1 and hp == 0:
                queue_proj(2)
            elif j == 2 and hp == 0:
                queue_proj(3)
            elif j == 2 and hp == 1:
                for ms in range(4):
                    unit_cproj(1, ms)
            if j == 3 and hp == 1:
                for ms in range(4):
                    unit_cproj(3, ms, ks=(0, 1))
            if j == 3 and hp == 3:
                drain_mode[0] = True
            pull(5000 if j == 3 else 900)
    for ms in range(4):
        unit_cproj(3, ms, run=True, ks=(2, 3), dst=out2_d)
    drain()



def _prep_inputs(x, w_attn, b_attn, w_proj):
    """Host-side shard + transpose + quantize for each of the 8 cores."""
    in_maps = []
    for g in range(N_CORES):
        b, grp = divmod(g, 4)
        xT = np.ascontiguousarray(np.asarray(x[b], np.float32).T)

        q_rows = []
        for lh in Q_ORDER:
            gh = HL * grp + lh
            q_rows.extend(range(HS * gh, HS * gh + HS))
        k0 = NE + KROWS * grp
        v0 = NE + N_KV * HS + KROWS * grp
        rows = q_rows + list(range(k0, k0 + KROWS)) + list(range(v0, v0 + KROWS))
        wqkvT = np.ascontiguousarray(w_attn[rows, :].T)

        cols = []
        for lh in Q_ORDER:
            gh = HL * grp + lh
            cols.extend(range(HS * gh, HS * gh + HS))
        wpT = np.ascontiguousarray(w_proj[:, cols].T)

        bq = np.asarray(b_attn[q_rows], np.float32).reshape(4, P)
        bk = (np.asarray(b_attn[k0:k0 + KROWS], np.float32) / 8.0).reshape(1, P)

        in_maps.append({
            "xT8": np.ascontiguousarray(xT[:, TQ:]).astype(NPE4),
            "x0bf": np.ascontiguousarray(xT[:, 0:TQ]).astype(NPBF16),
            "w8": (SW * wqkvT).astype(NPE4),
            "wbf": wqkvT.astype(NPBF16),
            "wp8": (SW * wpT).astype(NPE4),
            "wpbf": wpT.astype(NPBF16),
            "bqk": np.concatenate([
                bq, bk, np.asarray(b_attn[k0:k0 + KROWS],
                                   np.float32).reshape(1, P)], axis=0),
        })
    return in_maps


def get_nc():
    if "nc" not in _CACHE:
        _CACHE["nc"] = _build_program()
    return _CACHE["nc"]


def kernel(x, w_attn, b_attn, w_proj, b_proj):
    x = np.asarray(x, np.float32)
    w_attn = np.asarray(w_attn, np.float32)
    b_attn = np.asarray(b_attn, np.float32)
    w_proj = np.asarray(w_proj, np.float32)
    b_proj = np.asarray(b_proj, np.float32)

    nc = get_nc()
    in_maps = _prep_inputs(x, w_attn, b_attn, w_proj)
    res = run_bass_kernel_spmd(nc, in_maps, core_ids=list(range(N_CORES)))

    # host "all-reduce" over the 4 head-group cores per batch + bias folds
    bv = b_attn[NE + N_KV * HS:]                      # [512] v bias
    bv_full = np.repeat(bv.reshape(N_KV, HS), N_HEAD // N_KV, axis=0).reshape(-1)
    delta = bv_full @ w_proj.T + b_proj               # [2048]
    out = np.zeros((B, T, C), np.float32)
    for g in range(N_CORES):
        b = g // 4
        out[b] += np.asarray(res.results[g]["out"], np.float32)
        out[b, (NJ - 1) * TQ:] += np.asarray(res.results[g]["out2"], np.float32)
    out += delta[None, None, :]
    return out


# revision 15
# speedup vs baseline: 1.0092x; 1.0092x over previous
"""Trainium2 Bass kernel for GQA causal self-attention — fp8 DoubleRow version.

Model (hardcoded): B=2, T=2048, C=2048, n_head=32, n_kv=8, hs=64
Sharding: core g = (batch g//4, head-group g%4), 8 q-heads / 2 kv-heads per core.

Precision plan (error budget 2e-2 max-rel):
 - QKV (token windows 1-3) and c_proj (rows 512+) run in fp8e4m3 with
   DoubleRow perf mode: two 128-deep contraction tiles per instruction at
   0.5 cycles/row (4x bf16).  Weights are host-scaled by 16 to clear the
   e4m3 subnormal floor; inverse scales fold into the psum evacuations.
 - Scores for windows 2-3 run as zero-half DoubleRow on fp8 q/kT twins
   (2x PE rate; the DR pair's second slot points at a zeroed block).
   Window 0-1 scores, softmax and PV stay bf16: raw scores reach ~77
   (structured outliers), so the exp output needs bf16 range.
 - Tier-0 accuracy: fp8 noise is amplified where softmax averages few
   values (sum w^2 ~ 1), i.e. early query tokens.  Tokens 0:512 (= window
   0) get a fully bf16 path: window-0 projections from bf16 inputs, bf16
   attention (the normal code path), and bf16 c_proj for output rows
   0:512.  Later windows ride the fp8 projections; their y is quantized to
   fp8 (x8 scale) for the DR c_proj.

Layout/schedule bones are inherited from the bf16 kernel: K-stationary
scores (S^T tiles), pair-of-heads per matmul on PE rows 0:63/64:127, PV
with a ones-column for the softmax denominator, block-causal skipping, and
a micro-task queue that feeds projection/c_proj fillers to the PE between
attention steps.
"""

import sys
from collections import deque

import numpy as np
import ml_dtypes
from contextlib import ExitStack

for _p in ("/opt/trn_rl_repo", "/root/.axon_site/_ro/trn_rl_repo"):
    if _p not in sys.path:
        sys.path.append(_p)

import concourse.mybir as mybir
import concourse.tile as tile
from concourse import bacc
from concourse.bass_utils import run_bass_kernel_spmd

BF16 = mybir.dt.bfloat16
F32 = mybir.dt.float32
E4 = mybir.dt.float8e4
I16 = mybir.dt.int16
NPBF16 = ml_dtypes.bfloat16
NPE4 = ml_dtypes.float8_e4m3
DR = mybir.MatmulPerfMode.DoubleRow
LOG2E = 1.4426950408889634
# one-op Schraudolph exp on DVE: bf16 bits = s*128*log2e + (16256 - corr + .5)
SCH_A = 128.0 * LOG2E
SCH_B = 16256.0 - 0.458 * 128.0 + 0.5

B, T, C = 2, 2048, 2048
N_HEAD, N_KV, HS = 32, 8, 64
NE = 2048
N_CORES = 8
HL = 8          # q heads per core
KVL = 2         # kv heads per core
P = 128
TQ = 512        # tq tile (matmul moving width)
NJ = T // TQ    # 4 tq tiles
NT = T // P     # 16 token tiles
KC = C // P     # 16 contraction tiles over channels
QROWS = HL * HS          # 512 local q rows
KROWS = KVL * HS         # 128 local k rows
WCOLS = QROWS + 2 * KROWS  # 768 local w_attn rows
SW = 16.0       # host scale on w_attn / w_proj (fp8 subnormal avoidance)
SY = 8.0        # on-device scale on y before fp8

# position-block -> local head: q_sb m-tile mt rows [0:64]=head mt, [64:128]=head mt+4
Q_ORDER = [0, 4, 1, 5, 2, 6, 3, 7]

_CACHE = {}


def _build_program():
    nc = bacc.Bacc("TRN2", target_bir_lowering=False, debug=False)

    xT8_d = nc.dram_tensor("xT8", [C, T - TQ], E4, kind="ExternalInput")
    x0bf_d = nc.dram_tensor("x0bf", [C, TQ], BF16, kind="ExternalInput")
    w8_d = nc.dram_tensor("w8", [C, WCOLS], E4, kind="ExternalInput")
    wbf_d = nc.dram_tensor("wbf", [C, WCOLS], BF16, kind="ExternalInput")
    wp8_d = nc.dram_tensor("wp8", [QROWS, C], E4, kind="ExternalInput")
    wpbf_d = nc.dram_tensor("wpbf", [QROWS, C], BF16, kind="ExternalInput")
    bqk_d = nc.dram_tensor("bqk", [6, P], F32, kind="ExternalInput")
    out_d = nc.dram_tensor("out", [T, C], BF16, kind="ExternalOutput")
    # window-3 c_proj partial over k-tile pair (2,3) (head-pairs 2,3): summed
    # into out on the host.  Splitting it out lets the (0,1) pair of window
    # 3's c_proj overlap the last attention pairs instead of serializing.
    out2_d = nc.dram_tensor("out2", [TQ, C], BF16, kind="ExternalOutput")

    with tile.TileContext(nc) as tc:
        with ExitStack() as ctx:
            _emit(ctx, tc, nc, xT8_d, x0bf_d, w8_d, wbf_d, wp8_d, wpbf_d,
                  bqk_d, out_d, out2_d)
    nc.compile()
    return nc


def _emit(ctx, tc, nc, xT8_d, x0bf_d, w8_d, wbf_d, wp8_d, wpbf_d,
          bqk_d, out_d, out2_d):
    ExpF = mybir.ActivationFunctionType.Exp
    CopyF = mybir.ActivationFunctionType.Copy
    add = mybir.AluOpType.add
    mult = mybir.AluOpType.mult

    persist = ctx.enter_context(tc.tile_pool(name="persist", bufs=1))
    # PSUM budget (8 banks):
    #  pps: scores [128,1024] f32 (2 banks) x2 bufs          = 4 banks
    #  ppa: proj/cproj [128,512] f32 (1 bank) x2 bufs        = 2 banks
    #  ppo: PV accum [128,260] f32 / transpose [128,128]     = 2 banks
    pps = ctx.enter_context(tc.tile_pool(name="pps", bufs=2, space="PSUM"))
    ppa = ctx.enter_context(tc.tile_pool(name="ppa", bufs=2, space="PSUM"))
    ppo = ctx.enter_context(tc.tile_pool(name="ppo", bufs=2, space="PSUM"))
    ptpool = ctx.enter_context(tc.tile_pool(name="pt", bufs=9))
    rcpool = ctx.enter_context(tc.tile_pool(name="rc", bufs=12))
    ytpool = ctx.enter_context(tc.tile_pool(name="yt", bufs=6))
    mkpool = ctx.enter_context(tc.tile_pool(name="mk", bufs=2))
    outpool = ctx.enter_context(tc.tile_pool(name="os", bufs=8))

    # ---- persistent SBUF tensors ----
    xT8_sb = persist.tile([P, KC * (T - TQ)], E4, tag="xT8")
    x0_sb = persist.tile([P, KC * TQ], BF16, tag="x0")
    w8_sb = persist.tile([P, KC * WCOLS], E4, tag="w8")
    wbf_sb = persist.tile([P, KC * WCOLS], BF16, tag="wbf")
    wp8_sb = persist.tile([P, 4 * C], E4, tag="wp8")
    wpbf_sb = persist.tile([P, 4 * C], BF16, tag="wpbf")
    q_sb = persist.tile([P, 4 * T], BF16, tag="q")
    kT_sb = persist.tile([P, T], BF16, tag="k")
    # fp8 twins for the windows-2/3 zero-half DoubleRow scores (2x PE rate):
    # kT8 interleaves each [128,128] k-tile with a zero block (the DR
    # stationary's second slot); q8 holds windows 2-3 only, one real TQ
    # block + one zero TQ block per (mt, window) for the moving operand.
    kT8_sb = persist.tile([P, 2 * T], E4, tag="k8")
    q8_sb = persist.tile([P, 8 * 2 * TQ], E4, tag="q8")
    sc125 = persist.tile([P, 1], F32, tag="sc125")
    v_sb = persist.tile([P, NT * 130], BF16, tag="v")
    y8_sb = persist.tile([P, 4 * T], E4, tag="y8")
    y0_sb = persist.tile([P, 4 * TQ], BF16, tag="y0")
    bqk_sb = persist.tile([P, 6], F32, tag="bqk")
    ident = persist.tile([P, P], BF16, tag="ident")
    # single triangular mask: maskt[x, y] = 1 if y >= x else 0 (bf16)
    maskt = persist.tile([P, P], BF16, tag="maskt")

    # ---- input DMAs ----
    # dma_start costs ~650ns on the issuing sequencer and ~630ns on HWDGE,
    # so transfers are merged into few issues, emitted in consumption order.
    # The stream is sliced by priority: window-0's kT/v/q0 inputs first (ACT
    # exp work — the bottleneck — starts with the first attention pair),
    # then the q-mt1-3 weight slices, then window-1's fp8 inputs, etc.
    x0v = x0bf_d.ap().rearrange("(k p) t -> p k t", p=P)
    x0s = x0_sb.rearrange("p (k t) -> p k t", t=TQ)
    wbv = wbf_d.ap().rearrange("(k p) c -> p k c", p=P)
    wbs = wbf_sb.rearrange("p (k c) -> p k c", c=WCOLS)
    nc.sync.dma_start(x0s[:, 0:2, :], x0v[:, 0:2, :])
    nc.sync.dma_start(wbs[:, 0:2, QROWS:], wbv[:, 0:2, QROWS:])
    nc.sync.dma_start(wbs[:, 0:2, 0:P], wbv[:, 0:2, 0:P])
    for k0, k1 in ((2, 6), (6, 11), (11, 16)):
        nc.sync.dma_start(x0s[:, k0:k1, :], x0v[:, k0:k1, :])
        nc.sync.dma_start(wbs[:, k0:k1, QROWS:], wbv[:, k0:k1, QROWS:])
        nc.sync.dma_start(wbs[:, k0:k1, 0:P], wbv[:, k0:k1, 0:P])
    # biases (needed only at first evac, ~13us in)
    nc.sync.dma_start(bqk_sb[:], bqk_d.ap().rearrange("t p -> p t"))
    # q-mt1..3 weight slices (window-0 pairs 1-3)
    for k0, k1 in ((0, 8), (8, 16)):
        nc.sync.dma_start(wbs[:, k0:k1, P:QROWS], wbv[:, k0:k1, P:QROWS])
    # fp8 inputs for windows 1-3 (xT8 column n is token TQ+n), again sliced
    # so window-1's first pair (k/q0/v units) unblocks earliest
    xv8 = xT8_d.ap().rearrange("(k p) t -> p k t", p=P)
    wv8 = w8_d.ap().rearrange("(k p) c -> p k c", p=P)
    xs8 = xT8_sb.rearrange("p (k t) -> p k t", t=T - TQ)
    ws8 = w8_sb.rearrange("p (k c) -> p k c", c=WCOLS)
    nc.sync.dma_start(ws8[:, :, QROWS:], wv8[:, :, QROWS:])
    for k0, k1 in ((0, 8), (8, 16)):
        nc.sync.dma_start(xs8[:, k0:k1, 0:TQ], xv8[:, k0:k1, 0:TQ])
    nc.sync.dma_start(ws8[:, :, 0:P], wv8[:, :, 0:P])
    nc.sync.dma_start(ws8[:, :, P:QROWS], wv8[:, :, P:QROWS])
    for n in range(2, NJ):
        t0, t1 = (n - 1) * TQ, n * TQ
        for k0, k1 in ((0, 8), (8, 16)):
            nc.sync.dma_start(xs8[:, k0:k1, t0:t1], xv8[:, k0:k1, t0:t1])
    wpv8 = wp8_d.ap().rearrange("(k p) c -> p k c", p=P)
    wps8 = wp8_sb.rearrange("p (k c) -> p k c", c=C)
    for k0, k1 in ((0, 2), (2, 4)):
        nc.sync.dma_start(wps8[:, k0:k1, :], wpv8[:, k0:k1, :])
    wpvb = wpbf_d.ap().rearrange("(k p) c -> p k c", p=P)
    wpsb = wpbf_sb.rearrange("p (k c) -> p k c", c=C)
    for k0, k1 in ((0, 2), (2, 4)):
        nc.sync.dma_start(wpsb[:, k0:k1, :], wpvb[:, k0:k1, :])

    # ---- constants ----
    # warm the PE p-state ramp during the initial DMA wait (~3us of dummy
    # matmuls so the first real matmul runs at 2.4GHz).
    dmy = persist.tile([P, P], BF16, tag="dmy")
    nc.gpsimd.memset(dmy[:], 0.0)
    for w in range(26):
        pdmy = ppa.tile([P, P], F32, tag="pa", name="pdmy")
        nc.tensor.matmul(pdmy[:], dmy[:, 0:P], dmy[:, 0:P],
                         start=True, stop=True)
    mf = mkpool.tile([P, P], F32, tag="mf")
    nc.gpsimd.memset(mf[:], 1.0)
    nc.gpsimd.affine_select(
        out=mf[:], in_=mf[:], compare_op=mybir.AluOpType.is_ge,
        fill=0.0, base=0, pattern=[[1, P]], channel_multiplier=-1)
    nc.scalar.copy(maskt[:], mf[:])
    # identity for PE transposes
    mi = mkpool.tile([P, P], F32, tag="mi")
    nc.gpsimd.memset(mi[:], 1.0)
    nc.gpsimd.affine_select(
        out=mi[:], in_=mi[:], compare_op=mybir.AluOpType.is_equal,
        fill=0.0, base=0, pattern=[[1, P]], channel_multiplier=-1)
    nc.scalar.copy(ident[:], mi[:])
    nc.vector.memset(v_sb[:], 1.0)   # ones columns; data cols overwritten
    nc.vector.memset(sc125[:], 0.125)
    # zero the fp8 score-twin tensors (zero-half DR slots) on idle Pool
    nc.gpsimd.memset(kT8_sb[:], 0.0)
    nc.gpsimd.memset(q8_sb[:], 0.0)

    def xt8(kp, c0, n):    # xT8 k-pair AP [128, 2, n]; c0 in 0..T-TQ-1
        return xs8[:, kp:kp + 2, c0:c0 + n]

    def wq8(kp, mt):       # [128, 2, 128] q-weight pair (fp8)
        return ws8[:, kp:kp + 2, mt * P:(mt + 1) * P]

    def wk8(kp):
        return ws8[:, kp:kp + 2, QROWS:QROWS + P]

    def wv8t(kp):
        return ws8[:, kp:kp + 2, QROWS + P:QROWS + 2 * P]

    kT8v = kT8_sb.rearrange("p (i two c) -> p i two c", two=2, c=P)
    q8v = q8_sb.rearrange("p (g two t) -> p g two t", two=2, t=TQ)

    def x0t(k, c0, n):     # window-0 bf16 x tile
        return x0_sb[:, k * TQ + c0:k * TQ + c0 + n]

    def wbf_t(k, m0, n):   # bf16 w slice
        return wbf_sb[:, k * WCOLS + m0:k * WCOLS + m0 + n]

    # ---- micro-task queue ------------------------------------------------
    queue = deque()
    drain_mode = [False]
    drain_flip = [0]
    drain_pool = [0]
    vc = {"pe": 0.0, "act": 0.0}

    def pe_adv(ns):
        vc["pe"] += ns

    def pull(ns):
        while ns > 0 and queue:
            pe_ns, fn = queue.popleft()
            if fn is None:
                continue
            fn()
            vc["pe"] += pe_ns
            ns -= pe_ns

    def pull_to_act(margin=0.0):
        while vc["pe"] < vc["act"] + margin and queue:
            pe_ns, fn = queue.popleft()
            if fn is None:
                continue
            fn()
            vc["pe"] += pe_ns

    def drain():
        pull(float("inf"))

    def drain_to_marker():
        # pull until the next marker (end of a proj group) is consumed;
        # later fillers stay queued for in-window pulls
        while queue:
            pe_ns, fn = queue.popleft()
            if fn is None:
                return
            fn()
            vc["pe"] += pe_ns


    # ---- work units ------------------------------------------------------
    def unit_q(n, mt, pool=None, run=False):
        # fp8 DR q projection for window n>=1
        pool = pool or ppa
        st = {}
        micros = []
        c0 = (n - 1) * TQ

        def m_first():
            st["ps"] = pool.tile([P, TQ], F32, tag=_ptag(pool), name="psq")
            for kp in (0, 2, 4, 6):
                nc.tensor.matmul(st["ps"][:], wq8(kp, mt), xt8(kp, c0, TQ),
                                 start=(kp == 0), stop=False, perf_mode=DR)
        micros.append((430, m_first))

        def m_second():
            for kp in (8, 10, 12, 14):
                nc.tensor.matmul(st["ps"][:], wq8(kp, mt), xt8(kp, c0, TQ),
                                 start=False, stop=(kp == 14), perf_mode=DR)
            if n >= 2:
                nc.vector.tensor_scalar(
                    out=q8v[:, mt * 2 + (n - 2), 0, :],
                    in0=st["ps"][:], scalar1=1.0 / SW,
                    scalar2=bqk_sb[:, mt:mt + 1], op0=mult, op1=add)
            else:
                nc.vector.tensor_scalar(
                    out=q_sb[:, mt * T + n * TQ: mt * T + (n + 1) * TQ],
                    in0=st["ps"][:], scalar1=1.0 / SW,
                    scalar2=bqk_sb[:, mt:mt + 1], op0=mult, op1=add)
        micros.append((430, m_second))
        return _unit_done(micros, run)

    def unit_k(n, pool=None, run=False):
        pool = pool or ppa
        st = {}
        micros = []
        c0 = (n - 1) * TQ

        def m_first():
            st["ps"] = pool.tile([P, TQ], F32, tag=_ptag(pool), name="psk")
            for kp in (0, 2, 4, 6):
                nc.tensor.matmul(st["ps"][:], wk8(kp), xt8(kp, c0, TQ),
                                 start=(kp == 0), stop=False, perf_mode=DR)
        micros.append((430, m_first))

        def m_second():
            for kp in (8, 10, 12, 14):
                nc.tensor.matmul(st["ps"][:], wk8(kp), xt8(kp, c0, TQ),
                                 start=False, stop=(kp == 14), perf_mode=DR)
            nc.vector.tensor_scalar(
                out=kT_sb[:, n * TQ:(n + 1) * TQ],
                in0=st["ps"][:], scalar1=0.125 / SW, scalar2=bqk_sb[:, 4:5],
                op0=mult, op1=add)
            nc.vector.tensor_scalar(
                out=kT8v[:, 4 * n:4 * n + 4, 0, :],
                in0=st["ps"].rearrange("p (f c) -> p f c", c=P),
                scalar1=1.0 / SW, scalar2=bqk_sb[:, 5:6],
                op0=mult, op1=add)
        micros.append((430, m_second))
        return _unit_done(micros, run)

    def unit_v(i, pool=None, run=False):
        # v_sb tile i: [0:64]=kv0, 64=ones, [65:129]=kv1, 129=ones
        pool = pool or ppa
        micros = []
        c0 = (i - 4) * P

        def m_all():
            ps = pool.tile([P, P], F32, tag=_ptag(pool), name="psv")
            for kp in (0, 2, 4, 6, 8, 10, 12, 14):
                nc.tensor.matmul(ps[:, 0:P], xt8(kp, c0, P), wv8t(kp),
                                 start=(kp == 0), stop=(kp == 14),
                                 perf_mode=DR)
            nc.vector.tensor_scalar(out=v_sb[:, i * 130: i * 130 + 64],
                                    in0=ps[:, 0:64], scalar1=1.0 / SW,
                                    scalar2=None, op0=mult)
            nc.vector.tensor_scalar(out=v_sb[:, i * 130 + 65: i * 130 + 129],
                                    in0=ps[:, 64:128], scalar1=1.0 / SW,
                                    scalar2=None, op0=mult)
        micros.append((215, m_all))
        return _unit_done(micros, run)

    def unit_cproj(j, ms, run=False, ks=(0, 1, 2, 3), dst=None):
        # fp8 DR c_proj (j>=1): k-tile pairs (0,1)/(2,3); ks is (0,1,2,3),
        # (0,1) or (2,3)
        dst = dst if dst is not None else out_d
        row0 = (j * TQ if dst is out_d else 0) + ms * P
        y8v = y8_sb.rearrange("p (k t) -> p k t", t=T)
        wv = wp8_sb.rearrange("p (k c) -> p k c", c=C)
        pairs = [kp for kp in (0, 2) if kp in ks]
        micros = []
        ust = {}
        for n in range(NJ):
            st = {}

            def m_a(n=n, st=st):
                if drain_mode[0]:
                    drain_pool[0] ^= 1
                pool, tg = (pps, "ps") if drain_pool[0] else (ppa, "pa")
                st["pc"] = pool.tile([P, TQ], F32, tag=tg, name="pc")
                for pi, kp in enumerate(pairs):
                    nc.tensor.matmul(
                        st["pc"][:],
                        y8v[:, kp:kp + 2, j * TQ + ms * P: j * TQ + (ms + 1) * P],
                        wv[:, kp:kp + 2, n * TQ:(n + 1) * TQ],
                        start=(pi == 0), stop=(pi == len(pairs) - 1),
                        perf_mode=DR)
                _cproj_out(ust, st, n, dst, row0, 1.0 / (SY * SW))
            micros.append((108 * len(pairs), m_a))
        return _unit_done(micros, run)

    def unit_cproj0(j, ms, run=False):
        # bf16 c_proj for window-0 output rows (from y0_sb / wpbf)
        row0 = j * TQ + ms * P
        micros = []
        ust = {}
        for n in range(NJ):
            st = {}

            def m_a(n=n, st=st):
                if drain_mode[0]:
                    drain_pool[0] ^= 1
                pool, tg = (pps, "ps") if drain_pool[0] else (ppa, "pa")
                st["pc"] = pool.tile([P, TQ], F32, tag=tg, name="pc0")
                for k in (0, 1):
                    nc.tensor.matmul(
                        st["pc"][:], y0_sb[:, k * TQ + ms * P:k * TQ + (ms + 1) * P],
                        wpbf_sb[:, k * C + n * TQ:k * C + (n + 1) * TQ],
                        start=(k == 0), stop=False)

            def m_b(n=n, st=st):
                for k in (2, 3):
                    nc.tensor.matmul(
                        st["pc"][:], y0_sb[:, k * TQ + ms * P:k * TQ + (ms + 1) * P],
                        wpbf_sb[:, k * C + n * TQ:k * C + (n + 1) * TQ],
                        start=False, stop=(k == 3))
                _cproj_out(ust, st, n, out_d, row0, 1.0 / SY)
            micros.append((215, m_a))
            micros.append((215, m_b))
        return _unit_done(micros, run)

    def _cproj_out(ust, st, n, dst, row0, scl):
        # evacuate into a unit-wide staging tile; one bundled DMA per unit
        if n == 0:
            ust["os"] = outpool.tile([P, NJ * TQ], BF16, tag="os", name="os")
        drain_flip[0] ^= 1
        if drain_mode[0] and drain_flip[0]:
            # post-attention drain: ACT is idle; alternate psum evacuation
            # between DVE/ACT so the 2-slot psum ring doesn't serialize
            nc.scalar.activation(ust["os"][:, n * TQ:(n + 1) * TQ],
                                 st["pc"][:], CopyF, scale=scl)
        else:
            nc.vector.tensor_scalar(
                out=ust["os"][:, n * TQ:(n + 1) * TQ], in0=st["pc"][:],
                scalar1=scl, scalar2=None, op0=mult)
        if drain_mode[0] and n == 1:
            nc.scalar.dma_start(dst.ap()[row0: row0 + P, 0:2 * TQ],
                                ust["os"][:, 0:2 * TQ])
        elif drain_mode[0] and n == NJ - 1:
            nc.sync.dma_start(dst.ap()[row0: row0 + P, 2 * TQ:],
                              ust["os"][:, 2 * TQ:])
        elif n == NJ - 1:
            nc.sync.dma_start(dst.ap()[row0: row0 + P, :], ust["os"][:])

    def _ptag(pool):
        return {id(pps): "ps", id(ppa): "pa", id(ppo): "po"}[id(pool)]

    def _unit_done(micros, run):
        if run:
            for _, fn in micros:
                fn()
        else:
            queue.extend(micros)

    def _offload_exp(j, hp, i):
        # DVE-Schraudolph exp offload: disabled — measured error cost
        # (~1e-2, systematic linear-mantissa distortion) eats the budget
        return False

    # ---- attention -------------------------------------------------------
    def emit_score_mm(j, hp, i):
        # S^T tile [tk, tq] for tk-tile i of head pair hp, window j.
        # Head A occupies psum cols [c0:TQ], head B [TQ:2TQ-c0] (shifted
        # left by c0 so the pair-wide exp covers no dead columns).
        r = i - 4 * j
        c0 = max(0, r) * P
        qcol = hp * T + j * TQ
        ps = pps.tile([P, 2 * TQ], F32, tag="ps", name="pss")
        if j >= 2:
            # zero-half DoubleRow on the fp8 twins: 0.5 cycles/row
            g = hp * 2 + (j - 2)
            for h in (0, 1):
                rb = 64 * h
                nc.tensor.matmul(
                    ps[:, (c0 if h == 0 else TQ):
                       (TQ if h == 0 else 2 * TQ - c0)],
                    kT8v[rb:rb + 64, i, :, :],
                    q8v[rb:rb + 64, g, :, c0:TQ],
                    start=True, stop=True, perf_mode=DR)
            pe_adv(2 * (TQ - c0) * 0.2083)
        else:
            for h in (0, 1):
                rb = 64 * h
                nc.tensor.matmul(
                    ps[:, (c0 if h == 0 else TQ):
                       (TQ if h == 0 else 2 * TQ - c0)],
                    kT_sb[rb:rb + 64, i * P:(i + 1) * P],
                    q_sb[rb:rb + 64, qcol + c0: qcol + TQ],
                    start=True, stop=True)
            pe_adv(2 * (TQ - c0) * 0.417)
        return ps

    def attention(j, hp, pre_ps=None):
        # head pair (hp, hp+4): same q/y column tile `hp`, head A on
        # partitions 0:64 (kv0), head B on 64:128 (kv1).  Score matmuls are
        # software-pipelined one tile ahead of the exps so ACT (the
        # bottleneck engine) never waits on the PE; `pre_ps` carries the
        # pipelining across head-pair boundaries.
        nb = 4 * (j + 1)   # tk tiles in play (block-causal)
        mt = hp
        qcol = mt * T + j * TQ
        po = [ppo.tile([P, 260], F32, tag="po", name=f"po{g}")
              for g in range(2)]
        yts = []
        pres = list(pre_ps) if pre_ps else []
        ps_cur = pres.pop(0) if pres else emit_score_mm(j, hp, 0)
        ps_next_pair = []
        for i in range(nb):
            r = i - 4 * j
            roff = max(0, r)
            c0 = roff * P

            def bcol(h, c):
                # pt/psum column of chunk c for head h (B-shifted layout)
                return c * P if h == 0 else TQ + (c - roff) * P

            ps = ps_cur
            pt = ptpool.tile([P, 2 * TQ], BF16, tag="pt", name="pt")
            off = r < 0 and _offload_exp(j, hp, i)
            if off:
                # exp offloaded to DVE as a one-op Schraudolph to bf16 bits
                # (~3% weight error on this tile; ACT is the bottleneck)
                nc.vector.tensor_scalar(
                    out=pt[:, 0:2 * TQ].bitcast(I16), in0=ps[:, 0:2 * TQ],
                    scalar1=SCH_A, scalar2=SCH_B, op0=mult, op1=add)
            elif j >= 2:
                nc.scalar.activation(pt[:, c0:2 * TQ - c0],
                                     ps[:, c0:2 * TQ - c0], ExpF,
                                     scale=sc125[:, 0:1])
            else:
                nc.scalar.activation(pt[:, c0:2 * TQ - c0],
                                     ps[:, c0:2 * TQ - c0], ExpF)
            if i + 1 < nb:
                ps_cur = pres.pop(0) if pres else emit_score_mm(j, hp, i + 1)
            elif hp < 3:
                ps_next_pair.append(emit_score_mm(j, hp + 1, 0))
            if r >= 0:
                # diagonal boundary chunk: mask the [128,128] block per head
                # (on Pool: it is otherwise idle, and the masked chunk is
                # scheduled last in the PV order so the latency hides)
                for h in (0, 1):
                    nc.vector.tensor_tensor(
                        out=pt[:, bcol(h, r): bcol(h, r) + P],
                        in0=pt[:, bcol(h, r): bcol(h, r) + P],
                        in1=maskt[:], op=mult)
            if i == 0:
                cs = list(range(4))
            elif r >= 0:
                cs = list(range(r + 1, 4)) + [r]
            else:
                cs = list(range(4))
            for c in cs:
                cb = 130 * (c % 2)
                for h in (0, 1):
                    nc.tensor.matmul(
                        po[c // 2][:, cb + 65 * h: cb + 65 * h + 65],
                        pt[:, bcol(h, c): bcol(h, c) + P],
                        v_sb[:, i * 130 + 65 * h: i * 130 + 65 * h + 65],
                        start=(i == 0 and h == 0 and c in (0, 2)),
                        stop=(i == 4 * j + c), skip_group_check=True)
            if r >= 0:
                # chunk r's accumulation just stopped: normalize now so the
                # pair-end transposes only wait on the last chunk.
                yt = ytpool.tile([P, P], BF16, tag="yt", name="yt")
                cb = 130 * (r % 2)
                for h in (0, 1):
                    r_t = rcpool.tile([P, 1], F32, tag="rc", name="rc")
                    nc.vector.reciprocal(
                        r_t[:],
                        po[r // 2][:, cb + 65 * h + 64: cb + 65 * h + 65])
                    nc.vector.tensor_scalar(
                        out=yt[:, 64 * h:64 * h + 64],
                        in0=po[r // 2][:, cb + 65 * h: cb + 65 * h + 64],
                        scalar1=r_t[:, 0:1], scalar2=SY, op0=mult, op1=mult)
                yts.append(yt)
            if not off:
                vc["act"] = max(vc["act"], vc["pe"]) \
                    + (2 * TQ - 2 * c0) * 0.834 + 190
                vc["pe"] = max(vc["pe"], vc["act"])
            pe_adv((4 - max(0, r)) * 2 * 65 * 0.417)
            pull_to_act(500)
        pull(1400)
        # transpose [q, dA|dB] -> [dA|dB, q] into the c_proj layout; the
        # psum->sbuf copy converts bf16 -> fp8 for windows 1-3
        for c in range(4):
            tr = ppo.tile([P, P], BF16, tag="po", name="tr")
            nc.tensor.transpose(tr[:], yts[c][:], ident[:])
            if j == 0:
                nc.vector.tensor_copy(
                    y0_sb[:, mt * TQ + c * P: mt * TQ + (c + 1) * P], tr[:])
            else:
                nc.vector.tensor_copy(
                    y8_sb[:, qcol + c * P: qcol + (c + 1) * P], tr[:])
            if c < 3:
                pull(500)
        return ps_next_pair or None

    def unit_kbf(run=False):
        # accurate bf16 kT for tiles 0-3 (overwrites the fp8-derived boot
        # values once window 1 is done with them)
        st = {}
        micros = []

        def m_first():
            st["ps"] = ppa.tile([P, TQ], F32, tag="pa", name="pskb")
            for k in range(8):
                nc.tensor.matmul(st["ps"][:], wbf_t(k, QROWS, P),
                                 x0t(k, 0, TQ), start=(k == 0), stop=False)
        micros.append((430, m_first))

        def m_second():
            for k in range(8, KC):
                nc.tensor.matmul(st["ps"][:], wbf_t(k, QROWS, P),
                                 x0t(k, 0, TQ), start=False,
                                 stop=(k == KC - 1))
            nc.vector.tensor_scalar(
                out=kT_sb[:, 0:TQ], in0=st["ps"][:], scalar1=0.125,
                scalar2=bqk_sb[:, 4:5], op0=mult, op1=add)
            nc.vector.tensor_scalar(
                out=kT8v[:, 0:4, 0, :],
                in0=st["ps"].rearrange("p (f c) -> p f c", c=P),
                scalar1=bqk_sb[:, 5:6], scalar2=None, op0=add)
        micros.append((430, m_second))
        return _unit_done(micros, run)

    def unit_vbf(i, run=False):
        # accurate bf16 v tile i (0..3), overwriting the boot values
        micros = []

        def m_all():
            ps = ppa.tile([P, TQ], F32, tag="pa", name="psvb")
            for k in range(KC):
                nc.tensor.matmul(ps[:, 0:P], x0t(k, i * P, P),
                                 wbf_t(k, QROWS + P, P),
                                 start=(k == 0), stop=(k == KC - 1))
            nc.vector.tensor_copy(v_sb[:, i * 130: i * 130 + 64], ps[:, 0:64])
            nc.vector.tensor_copy(v_sb[:, i * 130 + 65: i * 130 + 129],
                                  ps[:, 64:128])
        micros.append((860, m_all))
        return _unit_done(micros, run)

    def unit_q0bf(mt, run=False):
        # window-0 bf16 q projection for m-tile mt (deferred from p0 so the
        # first attention pair starts as soon as q-mt0/kT/v are ready)
        st = {}
        micros = []

        def m_first():
            st["ps"] = ppa.tile([P, TQ], F32, tag="pa", name="psq0")
            for k in range(8):
                nc.tensor.matmul(st["ps"][:], wbf_t(k, mt * P, P),
                                 x0t(k, 0, TQ), start=(k == 0), stop=False)
        micros.append((430, m_first))

        def m_second():
            for k in range(8, KC):
                nc.tensor.matmul(st["ps"][:], wbf_t(k, mt * P, P),
                                 x0t(k, 0, TQ), start=False,
                                 stop=(k == KC - 1))
            nc.vector.tensor_scalar(
                out=q_sb[:, mt * T: mt * T + TQ], in0=st["ps"][:],
                scalar1=bqk_sb[:, mt:mt + 1], scalar2=None, op0=add)
        micros.append((430, m_second))
        return _unit_done(micros, run)

    # ---- schedule --------------------------------------------------------
    # p0: window-0 bf16 kT/q-mt0/v projections run directly, k-MAJOR so the
    # PE consumes input chunks in DMA-arrival order; q m-tiles 1-3 are
    # deferred into the filler queue so attention (and with it the ACT exp
    # stream, the bottleneck) starts as early as possible.
    p0q0 = pps.tile([P, TQ], F32, tag="ps", name="p0q0")
    p0k = ppo.tile([P, TQ], F32, tag="po", name="p0k")
    for k0 in range(0, KC, 2):
        for k in (k0, k0 + 1):
            nc.tensor.matmul(p0q0[:], wbf_t(k, 0, P), x0t(k, 0, TQ),
                             start=(k == 0), stop=(k == KC - 1))
        for k in (k0, k0 + 1):
            nc.tensor.matmul(p0k[:], wbf_t(k, QROWS, P), x0t(k, 0, TQ),
                             start=(k == 0), stop=(k == KC - 1))
    nc.vector.tensor_scalar(
        out=q_sb[:, 0:TQ], in0=p0q0[:],
        scalar1=bqk_sb[:, 0:1], scalar2=None, op0=add)
    nc.vector.tensor_scalar(
        out=kT_sb[:, 0:TQ], in0=p0k[:], scalar1=0.125,
        scalar2=bqk_sb[:, 4:5], op0=mult, op1=add)
    nc.vector.tensor_scalar(
        out=kT8v[:, 0:4, 0, :],
        in0=p0k.rearrange("p (f c) -> p f c", c=P),
        scalar1=bqk_sb[:, 5:6], scalar2=None, op0=add)
    # window-0 v tiles (bf16)
    for i, pool in zip(range(4), (ppo, pps, ppa, ppo)):
        ps = pool.tile([P, TQ], F32, tag=_ptag(pool), name="psv0")
        for k in range(KC):
            nc.tensor.matmul(ps[:, 0:P], x0t(k, i * P, P),
                             wbf_t(k, QROWS + P, P),
                             start=(k == 0), stop=(k == KC - 1))
        nc.vector.tensor_copy(v_sb[:, i * 130: i * 130 + 64], ps[:, 0:64])
        nc.vector.tensor_copy(v_sb[:, i * 130 + 65: i * 130 + 129],
                              ps[:, 64:128])

    def queue_proj(n):
        # kT / q-mt0 / v first: the next window's first attention pair only
        # needs those
        unit_k(n)
        unit_q(n, 0)
        for i in range(4 * n, 4 * n + 4):
            unit_v(i)
        queue.append((0, None))     # marker: first-pair prerequisites done
        for mt in (1, 2, 3):
            unit_q(n, mt)

    for mt in (1, 2, 3):
        unit_q0bf(mt)
    for j in range(NJ):
        if j > 0:
            # pull until this window's first-pair prerequisites (kT/q-mt0/v
            # of proj(j)) are emitted; q-mt1..3 stay queued for in-window
            # pulls
            drain_to_marker()
        if j == 3:
            for ms in range(4):
                unit_cproj0(0, ms)
            for ms in range(4):
                unit_cproj(2, ms)
        pre = None
        for hp in range(4):
            pre = attention(j, hp, pre_ps=pre)
            # fillers are queued only once their DMAs are in flight, so a
            # pulled micro never stalls the in-order PE stream (which would
            # stall the next score matmul and starve ACT)
            if j == 0 and hp == 2:
                queue_proj(1)
            elif j == 1 and hp == 0:
                queue_proj(2)
            elif j == 2 and hp == 0:
                queue_proj(3)
            elif j == 2 and hp == 1:
                for ms in range(4):
                    unit_cproj(1, ms)
            if j == 3 and hp == 1:
                for ms in range(4):
                    unit_cproj(3, ms, ks=(0, 1))
            if j == 3 and hp == 3:
                drain_mode[0] = True
            pull(5000 if j == 3 else 900)
    for ms in range(4):
        unit_cproj(3, ms, run=True, ks=(2, 3), dst=out2_d)
    drain()



def _prep_inputs(x, w_attn, b_attn, w_proj):
    """Host-side shard + transpose + quantize for each of the 8 cores."""
    in_maps = []
    for g in range(N_CORES):
        b, grp = divmod(g, 4)
        xT = np.ascontiguousarray(np.asarray(x[b], np.float32).T)

        q_rows = []
        for lh in Q_ORDER:
            gh = HL * grp + lh
            q_rows.extend(range(HS * gh, HS * gh + HS))
        k0 = NE + KROWS * grp
        v0 = NE + N_KV * HS + KROWS * grp
        rows = q_rows + list(range(k0, k0 + KROWS)) + list(range(v0, v0 + KROWS))
        wqkvT = np.ascontiguousarray(w_attn[rows, :].T)

        cols = []
        for lh in Q_ORDER:
            gh = HL * grp + lh
            cols.extend(range(HS * gh, HS * gh + HS))
        wpT = np.ascontiguousarray(w_proj[:, cols].T)

        bq = np.asarray(b_attn[q_rows], np.float32).reshape(4, P)
        bk = (np.asarray(b_attn[k0:k0 + KROWS], np.float32) / 8.0).reshape(1, P)

        in_maps.append({
            "xT8": np.ascontiguousarray(xT[:, TQ:]).astype(NPE4),
            "x0bf": np.ascontiguousarray(xT[:, 0:TQ]).astype(NPBF16),
            "w8": (SW * wqkvT).astype(NPE4),
            "wbf": wqkvT.astype(NPBF16),
            "wp8": (SW * wpT).astype(NPE4),
            "wpbf": wpT.astype(NPBF16),
            "bqk": np.concatenate([
                bq, bk, np.asarray(b_attn[k0:k0 + KROWS],
                                   np.float32).reshape(1, P)], axis=0),
        })
    return in_maps


def get_nc():
    if "nc" not in _CACHE:
        _CACHE["nc"] = _build_program()
    return _CACHE["nc"]


def kernel(x, w_attn, b_attn, w_proj, b_proj):
    x = np.asarray(x, np.float32)
    w_attn = np.asarray(w_attn, np.float32)
    b_attn = np.asarray(b_attn, np.float32)
    w_proj = np.asarray(w_proj, np.float32)
    b_proj = np.asarray(b_proj, np.float32)

    nc = get_nc()
    in_maps = _prep_inputs(x, w_attn, b_attn, w_proj)
    res = run_bass_kernel_spmd(nc, in_maps, core_ids=list(range(N_CORES)))

    # host "all-reduce" over the 4 head-group cores per batch + bias folds
    bv = b_attn[NE + N_KV * HS:]                      # [512] v bias
    bv_full = np.repeat(bv.reshape(N_KV, HS), N_HEAD // N_KV, axis=0).reshape(-1)
    delta = bv_full @ w_proj.T + b_proj               # [2048]
    out = np.zeros((B, T, C), np.float32)
    for g in range(N_CORES):
        b = g // 4
        out[b] += np.asarray(res.results[g]["out"], np.float32)
        out[b, (NJ - 1) * TQ:] += np.asarray(res.results[g]["out2"], np.float32)
    out += delta[None, None, :]
    return out
